# revision 24
# baseline (speedup 1.0000x reference)
"""TRN2 Bass kernel for nn_CrossAttention (sparse channel attention + prompt
fusion), sharded spatially over 8 NeuronCores (16 image rows + halo per core).

v2 design:
- fp8(e4m3) DoubleRow matmuls (2 K-subtiles per instruction) for the qkv 1x1
  conv, the 3x3 depthwise conv (as K=128 diagonal matmuls with vertically
  paired taps), attn@v and the prompt w1 conv.  Static power-of-2 scales keep
  everything out of the fp8 subnormal range; scales are divided out in the
  PSUM evacuations.
- proj is folded into attn@v:  out0 = (Wproj @ A_blockdiag) @ V, with
  MT = A^T Wproj^T computed once per batch on the PE (A assembled from the
  softmax output by same-partition block copies -- no transposes needed).
- the per-(batch) Gram AllReduces are split so each overlaps the other
  batch's compute.
"""
import sys

for _p in ("/opt/trn_rl_repo", "/root/.axon_site/_ro/trn_rl_repo"):
    if _p not in sys.path:
        sys.path.insert(0, _p)

import numpy as np

B, DIM, HEADS, Himg, Wimg = 2, 384, 8, 128, 128
C = DIM // HEADS            # 48
QKVC = 3 * DIM              # 1152
NCORE = 8
ROWS = Himg // NCORE        # 16 rows per core
NL = ROWS * Wimg            # 2048 local pixels
HR = ROWS + 2               # 18 rows with halo
NH = HR * Wimg              # 2304 halo pixels
PADW = 144                  # padded row pitch (16-aligned for DoubleRow)
NPAD = HR * PADW            # 2592
NT = 11                     # qkv out tiles: q 0-3 (head-padded), k 4-7, v 8-10
NSEG = 3                    # 384 attention rows per batch / 128
CH = 512
NCHUNK = NL // CH           # 4

# power-of-2 scales (see fp8_sim2.py numerics check)
S_X, S_WQ, S_QKV, S_WD = 16.0, 512.0, 32.0, 512.0
S_V, S_MT, S_O, S_W1 = 1024.0, 512.0, 256.0, 512.0
EV_QKV = S_QKV / (S_X * S_WQ)     # qkv psum -> fp8 qkvpad
EV_V = S_V / (S_QKV * S_WD)       # dw psum -> fp8 v
EV_QK = 1024.0 / (S_QKV * S_WD)   # dw psum -> bf16 q,k (scale cancels in norm)
EV_O = S_O / (S_MT * S_V)         # attn@v psum -> fp8 out0
EV_G = 1.0 / (S_O * S_W1)         # w1 psum -> true scale (gelu input)


def qkv_out_map():
    """raw qkv channel -> (tile, partition) in the head-padded layout."""
    perm = np.zeros((QKVC, 2), np.int64)
    for c in range(DIM):            # q
        h, j = c // C, c % C
        perm[c] = (h // 2, 64 * (h % 2) + j)
    for c in range(DIM):            # k
        h, j = c // C, c % C
        perm[DIM + c] = (4 + h // 2, 64 * (h % 2) + j)
    for j in range(DIM):            # v
        perm[2 * DIM + j] = (8 + j // 128, j % 128)
    return perm


def prep_constants(inputs):
    import ml_dtypes
    bf16 = ml_dtypes.bfloat16
    fp8 = ml_dtypes.float8_e4m3fn
    perm = qkv_out_map()
    w_qkv = np.asarray(inputs["w_qkv"], np.float32)
    w_dw = np.asarray(inputs["w_dw"], np.float32).reshape(QKVC, 9)
    w_proj = np.asarray(inputs["w_proj"], np.float32)

    # wqkvT8 [128, 3, NT*128]: permuted output channels, scaled
    wqkvT8 = np.zeros((128, 3, NT * 128), np.float32)
    for rc in range(QKVC):
        m, p = perm[rc]
        wqkvT8[:, :, m * 128 + p] = w_qkv[rc].reshape(3, 128).T * S_WQ
    # dwdiag8 [128, NT*9*128]: per (tile m, slot q) a diagonal block.
    # Slot order TAPORD puts the vertical DoubleRow pairs (dy=0,dy=1) at
    # adjacent slots 2*dx, 2*dx+1; the dy=2 singles at slots 6,7,8.
    TAPORD = [0, 3, 1, 4, 2, 5, 6, 7, 8]
    dwdiag8 = np.zeros((128, NT * 9 * 128), np.float32)
    for rc in range(QKVC):
        m, p = perm[rc]
        for q, tp in enumerate(TAPORD):
            dwdiag8[p, (m * 9 + q) * 128 + p] = w_dw[rc, tp] * S_WD

    def lhsT3(w):   # [out, 384] -> [3, 128, out]
        return np.transpose(np.asarray(w, np.float32).reshape(-1, 3, 128),
                            (1, 2, 0)).copy()

    def lhsT3f(w, s):  # [out, 384] -> [128, 3, out] scaled
        return np.ascontiguousarray(
            np.transpose(np.asarray(w, np.float32).reshape(-1, 3, 128),
                         (2, 1, 0)) * s)

    w2pad = {}
    for br, key in (("chr", "chr_w2"), ("detg", "detg_w2")):
        w2 = np.asarray(inputs[key], np.float32)       # [1, 384]
        pad = np.zeros((128, 3, 16), np.float32)
        pad[:, :, 0] = w2.reshape(3, 128).T
        w2pad[br] = pad

    sel = np.zeros((HEADS, NSEG * 128), np.float32)
    for s in range(NSEG):
        for p in range(128):
            sel[(128 * s + p) // C, s * 128 + p] = 1.0

    # block-diag assembly mask: mask3[p, ct, C*h+d] = ((128*ct+p)//C == h)
    mask3 = np.zeros((128, 3, DIM), np.float32)
    for ct in range(3):
        for p in range(128):
            h = (128 * ct + p) // C
            mask3[p, ct, C * h:C * (h + 1)] = 1.0

    out = dict(
        wqkvT8=wqkvT8.astype(fp8),
        dwdiag8=dwdiag8.astype(fp8),
        wprojT=lhsT3(w_proj).astype(bf16),                       # [3,128,384]
        w1T8_chr=lhsT3f(inputs["chr_w1"], S_W1).astype(fp8),     # [128,3,384]
        w1T8_detg=lhsT3f(inputs["detg_w1"], S_W1).astype(fp8),
        w2p_chr=w2pad["chr"].astype(bf16),                       # [128,3,16]
        w2p_detg=w2pad["detg"].astype(bf16),
        wtT_chr=np.asarray(inputs["chr_wt"], np.float32).T.copy().astype(bf16),
        wtT_detg=np.asarray(inputs["detg_wt"], np.float32).T.copy().astype(bf16),
        b1_chr=np.ascontiguousarray(np.asarray(inputs["chr_b1"], np.float32).reshape(3, 128).T),
        b1_detg=np.ascontiguousarray(np.asarray(inputs["detg_b1"], np.float32).reshape(3, 128).T),
        bt_chr=np.ascontiguousarray(np.asarray(inputs["chr_bt"], np.float32).reshape(3, 128).T),
        bt_detg=np.ascontiguousarray(np.asarray(inputs["detg_bt"], np.float32).reshape(3, 128).T),
        b2s=np.asarray([float(np.asarray(inputs["chr_b2"]).ravel()[0]),
                        float(np.asarray(inputs["detg_b2"]).ravel()[0])],
                       np.float32).reshape(1, 2),
        temp8=np.asarray(inputs["temperature"], np.float32).reshape(HEADS, 1).copy(),
        attns2=np.asarray(inputs["attns"], np.float32).reshape(1, 2).copy(),
        detg_z=np.asarray(inputs["detg_z"], np.float32).reshape(1, 64).copy(),
        sel3=sel,
        mask3=mask3.astype(bf16),
        ones1f=np.ones((1, 128), np.float32),
        ones64f=np.ones((64, 128), np.float32),
        ones1b=np.ones((1, 64), np.float32).astype(bf16),
    )
    return out


def shard_inputs(inputs, consts):
    import ml_dtypes
    bf16 = ml_dtypes.bfloat16
    fp8 = ml_dtypes.float8_e4m3fn
    x = np.asarray(inputs["x"], np.float32)
    gk0 = np.asarray(inputs["gk0"], np.float32)
    gk1 = np.asarray(inputs["gk1"], np.float32)
    xp = np.pad(x, ((0, 0), (0, 0), (1, 1), (0, 0)))
    maps = []
    for ci in range(NCORE):
        r0 = ROWS * ci
        xs = xp[:, :, r0:r0 + HR, :].reshape(B, 3, 128, NH) * S_X
        g0 = gk0[:, :, r0:r0 + ROWS, :].reshape(B, 64, NL)
        g1 = gk1[:, :, r0:r0 + ROWS, :].reshape(B, 64, NL)
        m = {"x_s": np.ascontiguousarray(xs).astype(fp8),
             "gk0_s": np.ascontiguousarray(g0).astype(bf16),
             "gk1_s": np.ascontiguousarray(g1).astype(bf16)}
        m.update(consts)
        maps.append(m)
    return maps


from contextlib import ExitStack

import concourse.bass as bass
import concourse.tile as tile
import concourse.mybir as mybir
from concourse import bacc

f32 = mybir.dt.float32
bf16 = mybir.dt.bfloat16
fp8 = mybir.dt.float8e4
AX = mybir.AxisListType
OP = mybir.AluOpType
AF = mybir.ActivationFunctionType
DR = mybir.MatmulPerfMode.DoubleRow


def build_program():
    nc = bacc.Bacc("TRN2", debug=False, num_devices=NCORE,
                   target_bir_lowering=False)

    def din(name, shape, dt):
        return nc.dram_tensor(name, list(shape), dt, kind="ExternalInput").ap()

    t = {}
    t["x_s"] = din("x_s", (B, 3, 128, NH), fp8)
    t["gk0_s"] = din("gk0_s", (B, 64, NL), bf16)
    t["gk1_s"] = din("gk1_s", (B, 64, NL), bf16)
    t["wqkvT8"] = din("wqkvT8", (128, 3, NT * 128), fp8)
    t["dwdiag8"] = din("dwdiag8", (128, NT * 9 * 128), fp8)
    t["wprojT"] = din("wprojT", (3, 128, DIM), bf16)
    for br in ("chr", "detg"):
        t[f"w1T8_{br}"] = din(f"w1T8_{br}", (128, 3, DIM), fp8)
        t[f"w2p_{br}"] = din(f"w2p_{br}", (128, 3, 16), bf16)
        t[f"wtT_{br}"] = din(f"wtT_{br}", (64, DIM), bf16)
        t[f"b1_{br}"] = din(f"b1_{br}", (128, 3), f32)
        t[f"bt_{br}"] = din(f"bt_{br}", (128, 3), f32)
    t["b2s"] = din("b2s", (1, 2), f32)
    t["temp8"] = din("temp8", (HEADS, 1), f32)
    t["attns2"] = din("attns2", (1, 2), f32)
    t["detg_z"] = din("detg_z", (1, 64), f32)
    t["sel3"] = din("sel3", (HEADS, NSEG * 128), f32)
    t["mask3"] = din("mask3", (128, 3, DIM), bf16)
    t["ones1f"] = din("ones1f", (1, 128), f32)
    t["ones64f"] = din("ones64f", (64, 128), f32)
    t["ones1b"] = din("ones1b", (1, 64), bf16)

    t["OUT"] = nc.dram_tensor("OUT", [B, 3, 128, NL], f32, kind="ExternalOutput").ap()

    for b in range(B):
        t[f"g3_part{b}"] = nc.dram_tensor(f"g3_part{b}", [HEADS, 96, 96], f32)
        t[f"g3_all{b}"] = nc.dram_tensor(f"g3_all{b}", [HEADS, 96, 96], f32,
                                         addr_space="Shared")
        t[f"qk_dense{b}"] = nc.dram_tensor(f"qk_dense{b}", [NSEG * 128, C], f32)
        t[f"rq_flat{b}"] = nc.dram_tensor(f"rq_flat{b}", [NSEG * 128], f32)
    t["zb"] = nc.dram_tensor("zb", [64], f32)

    with tile.TileContext(nc) as tc, ExitStack() as ctx:
        _body(tc, ctx, t)
    nc.compile()
    return nc


def _body(tc, ctx, t):
    nc = tc.nc
    wp = ctx.enter_context(tc.tile_pool(name="wp", bufs=1))
    pp = ctx.enter_context(tc.tile_pool(name="pp", bufs=1))
    sp = ctx.enter_context(tc.tile_pool(name="sp", bufs=1))
    ps_pool = ctx.enter_context(tc.tile_pool(name="ps", bufs=2, space="PSUM"))

    ec = [0]

    def evac(out_ap, in_ap, scale):
        """PSUM evacuation with scale, alternating scalar/vector."""
        if ec[0] % 3 == 0:
            nc.scalar.activation(out_ap, in_ap, AF.Identity, bias=0.0, scale=scale)
        else:
            nc.vector.tensor_scalar_mul(out_ap, in_ap, scale)
        ec[0] += 1

    # ---------------- constants into SBUF ----------------
    # x first -- the first qkv matmul needs only x and wqv
    x_sb = [wp.tile([128, 3, NH], fp8, tag=f"x{b}", name=f"x{b}") for b in range(B)]
    for b in range(B):
        nc.sync.dma_start(x_sb[b][:], t["x_s"][b].rearrange("k p s -> p k s"))
    wqv = wp.tile([128, 3, NT * 128], fp8, tag="wqv", name="wqv")
    nc.sync.dma_start(wqv[:], t["wqkvT8"])
    dwd = wp.tile([128, NT * 9 * 128], fp8, tag="dwd", name="dwd")
    nc.sync.dma_start(dwd[:], t["dwdiag8"])
    wprojT = [wp.tile([128, DIM], bf16, tag=f"wproj{k}", name=f"wproj{k}") for k in range(3)]
    for k in range(3):
        nc.sync.dma_start(wprojT[k][:], t["wprojT"][k])
    w1T8, w2p, wtT, b1, bt = {}, {}, {}, {}, {}
    for br in ("chr", "detg"):
        w1T8[br] = wp.tile([128, 3, DIM], fp8, tag=f"w1{br}", name=f"w1{br}")
        nc.sync.dma_start(w1T8[br][:], t[f"w1T8_{br}"])
        w2p[br] = wp.tile([128, 3, 16], bf16, tag=f"w2{br}", name=f"w2{br}")
        nc.sync.dma_start(w2p[br][:], t[f"w2p_{br}"])
        wtT[br] = wp.tile([64, DIM], bf16, tag=f"wt{br}", name=f"wt{br}")
        nc.sync.dma_start(wtT[br][:], t[f"wtT_{br}"])
        b1[br] = wp.tile([128, 3], f32, tag=f"b1{br}", name=f"b1{br}")
        nc.sync.dma_start(b1[br][:], t[f"b1_{br}"])
        bt[br] = wp.tile([128, 3], f32, tag=f"bt{br}", name=f"bt{br}")
        nc.sync.dma_start(bt[br][:], t[f"bt_{br}"])
    b2sb = wp.tile([1, 2], f32, tag="b2", name="b2")
    nc.sync.dma_start(b2sb[:], t["b2s"])
    tempc = wp.tile([HEADS, 1], f32, tag="temp", name="temp")
    nc.sync.dma_start(tempc[:], t["temp8"])
    attns_sb = wp.tile([1, 2], f32, tag="attns", name="attns")
    nc.sync.dma_start(attns_sb[:], t["attns2"])
    zrow = wp.tile([1, 64], f32, tag="zrow", name="zrow")
    nc.sync.dma_start(zrow[:], t["detg_z"])
    sel3 = wp.tile([HEADS, NSEG * 128], f32, tag="sel3", name="sel3")
    nc.sync.dma_start(sel3[:], t["sel3"])
    mask3 = wp.tile([128, 3, DIM], bf16, tag="mask3", name="mask3")
    nc.sync.dma_start(mask3[:], t["mask3"])
    ones1f = wp.tile([1, 128], f32, tag="ones1f", name="ones1f")
    nc.sync.dma_start(ones1f[:], t["ones1f"])
    ones64f = wp.tile([64, 128], f32, tag="ones64f", name="ones64f")
    nc.sync.dma_start(ones64f[:], t["ones64f"])
    ones1b = wp.tile([1, 64], bf16, tag="ones1b", name="ones1b")
    nc.sync.dma_start(ones1b[:], t["ones1b"])
    gk_sb = {}
    for b in range(B):
        for br, gk_d in (("chr", t["gk0_s"]), ("detg", t["gk1_s"])):
            g = wp.tile([64, NL], bf16, tag=f"gk{b}{br}", name=f"gk{b}{br}")
            nc.sync.dma_start(g[:], gk_d[b])
            gk_sb[(b, br)] = g

    # per-batch persistent: v (fp8, channel-major) tiles
    vcm8 = [wp.tile([128, 3, NL], fp8, tag=f"v{b}", name=f"v{b}") for b in range(B)]
    dwdv = dwd[:].rearrange("p (m t c) -> p m t c", t=9, c=128)

    # ================= per-batch compute: qkv, dw, gram =================
    for b in range(B):
        qkvpad = pp.tile([128, NT * NPAD], fp8, tag="bigA", name=f"qkvpad{b}")
        pvm = qkvpad[:].rearrange("p (m s) -> p m s", s=NPAD)
        xv = x_sb[b][:]
        qk_cm = [pp.tile([128, NL], bf16, tag=f"u{m}", name=f"qk{b}_{m}")
                 for m in range(8)]

        def qkv_tile(m):
            pvr = pvm[:, m, :].rearrange("p (r w) -> p r w", w=PADW)
            nc.vector.memset(pvr[:, :, 0:1], 0.0)
            nc.vector.memset(pvr[:, :, 129:130], 0.0)
            for nck in range(6):        # 6 x 384px (3 rows) over 18 halo rows
                psq = ps_pool.tile([128, 384], f32, tag=f"dw{nck % 2}", name="qkvps")
                nc.tensor.matmul(psq[:], wqv[:, 0:2, m * 128:(m + 1) * 128],
                                 xv[:, 0:2, nck * 384:(nck + 1) * 384],
                                 perf_mode=DR, start=True, stop=False)
                nc.tensor.matmul(psq[:], wqv[:, 2, m * 128:(m + 1) * 128],
                                 xv[:, 2, nck * 384:(nck + 1) * 384],
                                 start=False, stop=True)
                evac(pvr[:, 3 * nck:3 * nck + 3, 1:129],
                     psq[:].rearrange("p (r w) -> p r w", w=128), EV_QKV)

        def dw_tile(m):
            pvr = pvm[:, m, :].rearrange("p (r w) -> p r w", w=PADW)
            for yg in range(4):                  # groups of 4 output rows
                psd = ps_pool.tile([128, 512], f32, tag=f"dw{2 + yg % 2}", name="dwps")
                for yl in range(4):
                    y = 4 * yg + yl
                    out = psd[:, 128 * yl:128 * yl + 128]
                    for dx in range(3):
                        nc.tensor.matmul(
                            out, dwdv[:, m, 2 * dx:2 * dx + 2, :],
                            pvr[:, y:y + 2, dx:dx + 128],
                            perf_mode=DR, start=(dx == 0), stop=False,
                            skip_group_check=True)
                    for dx in range(3):
                        nc.tensor.matmul(
                            out, dwdv[:, m, 6 + dx, :],
                            pvr[:, y + 2, dx:dx + 128],
                            start=False, stop=(dx == 2),
                            skip_group_check=True)
                if m < 8:
                    evac(qk_cm[m][:, 512 * yg:512 * (yg + 1)], psd[:], EV_QK)
                else:
                    evac(vcm8[b][:, m - 8, 512 * yg:512 * (yg + 1)], psd[:], EV_V)

        # q,k tiles first so the Gram AllReduce can start before the v work
        for m in range(8):
            qkv_tile(m)
        for m in range(8):
            dw_tile(m)

        # ---- transpose q,k to pixel-major, stacked per head ----
        s_pm = pp.tile([128, 16 * 768], bf16, tag="spm", name=f"spm{b}")
        spm3 = s_pm[:].rearrange("p (c blk) -> p c blk", blk=768)
        for h in range(HEADS):
            qb = 64 * (h % 2)
            nc.sync.dma_start_transpose(
                spm3[:, :, 96 * h:96 * h + 48], qk_cm[h // 2][qb:qb + 48, :])
            nc.sync.dma_start_transpose(
                spm3[:, :, 96 * h + 48:96 * h + 96], qk_cm[4 + h // 2][qb:qb + 48, :])

        # ---- stacked Gram [96,96] per head ----
        g3sb = pp.tile([96, 8 * 96], f32, tag="g3sb", name=f"g3sb{b}")
        for h in range(HEADS):
            psg = ps_pool.tile([96, 96], f32, tag=f"dw{h % 4}", name="g3ps")
            for ckk in range(16):
                lhs = spm3[:, ckk, 96 * h:96 * h + 96]
                nc.tensor.matmul(psg[:], lhs, lhs,
                                 start=(ckk == 0), stop=(ckk == 15))
            nc.vector.tensor_copy(g3sb[:, 96 * h:96 * (h + 1)], psg[:])
        nc.sync.dma_start(
            t[f"g3_part{b}"].ap().rearrange("h r c -> r h c"),
            g3sb[:].rearrange("r (h c) -> r h c", c=96))

        nc.gpsimd.collective_compute(
            "AllReduce", OP.add, replica_groups=[list(range(NCORE))],
            ins=[t[f"g3_part{b}"].ap().opt()],
            outs=[t[f"g3_all{b}"].ap().opt()])

        # v tiles after the AllReduce is queued
        for m in range(8, NT):
            qkv_tile(m)
        for m in range(8, NT):
            dw_tile(m)

    # attns broadcast [128, 2]
    ps_a = ps_pool.tile([128, 2], f32, tag="dw0", name="attnsps")
    nc.tensor.matmul(ps_a[:], ones1f[:], attns_sb[:], start=True, stop=True)
    attns_bc = wp.tile([128, 2], f32, tag="attnsbc", name="attnsbc")
    nc.vector.tensor_copy(attns_bc[:], ps_a[:])

    # z-bar [64, 128] bf16 (for alpha)
    zsq = sp.tile([1, 64], f32, tag="zsq", name="zsq")
    nc.scalar.square(zsq[:], zrow[:])
    zss = sp.tile([1, 1], f32, tag="zss", name="zss")
    nc.vector.reduce_sum(zss[:], zsq[:], axis=AX.X)
    nc.scalar.sqrt(zss[:], zss[:])
    nc.vector.tensor_scalar_max(zss[:], zss[:], 1e-12)
    zrs = sp.tile([1, 1], f32, tag="zrs", name="zrs")
    nc.vector.reciprocal(zrs[:], zss[:])
    zn = sp.tile([1, 64], f32, tag="zn", name="zn")
    nc.vector.tensor_scalar_mul(zn[:], zrow[:], zrs[:, 0:1])
    nc.sync.dma_start(t["zb"].ap().rearrange("(a b) -> a b", a=1), zn[:])
    zcol = sp.tile([64, 1], f32, tag="zcol", name="zcol")
    nc.sync.dma_start(zcol[:], t["zb"].ap().rearrange("(p a) -> p a", a=1))
    zrep = sp.tile([64, 128], f32, tag="zrep", name="zrep")
    nc.vector.tensor_scalar_mul(zrep[:], ones64f[:], zcol[:, 0:1])
    zrep16 = wp.tile([64, 128], bf16, tag="zrep16", name="zrep16")
    nc.vector.tensor_copy(zrep16[:], zrep[:])


    # ================= per-batch attention matrix + output chain ========
    for b in range(B):
        g3a = t[f"g3_all{b}"]
        # norms from the stacked-Gram diagonals
        norm2 = sp.tile([HEADS, 96], f32, tag="norm2", name="norm2")
        nc.sync.dma_start(norm2[:, 0:48],
                          bass.AP(tensor=g3a, offset=0, ap=[[96 * 96, HEADS], [97, 48]]))
        nc.sync.dma_start(norm2[:, 48:96],
                          bass.AP(tensor=g3a, offset=48 * 96 + 48,
                                  ap=[[96 * 96, HEADS], [97, 48]]))
        nc.scalar.sqrt(norm2[:], norm2[:])
        nc.vector.tensor_scalar_max(norm2[:], norm2[:], 1e-12)
        rn = sp.tile([HEADS, 96], f32, tag="rn", name="rn")
        nc.vector.reciprocal(rn[:], norm2[:])
        rqf = sp.tile([HEADS, 48], f32, tag="rqf", name="rqf")
        nc.vector.tensor_scalar_mul(rqf[:], rn[:, 0:48], tempc[:, 0:1])
        nc.sync.dma_start(t[f"rq_flat{b}"].ap().rearrange("(a c) -> a c", a=HEADS), rqf[:])
        rq_seg = sp.tile([128, NSEG], f32, tag="rqseg", name="rqseg")
        nc.sync.dma_start(rq_seg[:],
                          t[f"rq_flat{b}"].ap().rearrange("(s p) -> p s", s=NSEG))
        psrk = ps_pool.tile([128, NSEG * 48], f32, tag="dw0", name="rkps")
        for s in range(NSEG):
            nc.tensor.matmul(psrk[:, 48 * s:48 * s + 48],
                             sel3[:, 128 * s:128 * s + 128], rn[:, 48:96],
                             start=True, stop=True)
        rk_bc = sp.tile([128, NSEG * 48], f32, tag="rkbc", name="rkbc")
        nc.vector.tensor_copy(rk_bc[:], psrk[:])
        nc.sync.dma_start(
            t[f"qk_dense{b}"].ap().rearrange("(h c) d -> h c d", h=HEADS),
            g3a.ap()[:, 0:48, 48:96])
        G_seg = sp.tile([128, NSEG * 48], f32, tag="gseg", name="gseg")
        nc.sync.dma_start(G_seg[:].rearrange("p (s d) -> p s d", s=NSEG),
                          t[f"qk_dense{b}"].ap().rearrange("(s p) d -> p s d", s=NSEG))

        A = sp.tile([128, NSEG * 48], f32, tag="A", name="A")
        seg = lambda tl, s: tl[:, 48 * s:48 * s + 48]
        for s in range(NSEG):
            nc.vector.scalar_tensor_tensor(
                out=seg(A, s), in0=seg(G_seg, s), scalar=rq_seg[:, s:s + 1],
                in1=seg(rk_bc, s), op0=OP.mult, op1=OP.mult)

        m1 = sp.tile([128, NSEG * 8], f32, tag="m1", name="m1")
        m2 = sp.tile([128, NSEG * 8], f32, tag="m2", name="m2")
        m3 = sp.tile([128, NSEG * 8], f32, tag="m3", name="m3")
        At1 = sp.tile([128, NSEG * 48], f32, tag="At1", name="At1")
        At2 = sp.tile([128, NSEG * 48], f32, tag="At2", name="At2")
        for s in range(NSEG):
            nc.vector.max(m1[:, 8 * s:8 * s + 8], seg(A, s))
            nc.vector.match_replace(seg(At1, s), m1[:, 8 * s:8 * s + 8], seg(A, s), -1e30)
            nc.vector.max(m2[:, 8 * s:8 * s + 8], seg(At1, s))
            nc.vector.match_replace(seg(At2, s), m2[:, 8 * s:8 * s + 8], seg(At1, s), -1e30)
            nc.vector.max(m3[:, 8 * s:8 * s + 8], seg(At2, s))

        rowst = sp.tile([128, NSEG], f32, tag="rowst", name="rowst")
        nc.vector.reduce_max(rowst[:], m1[:].rearrange("p (s e) -> p s e", e=8), axis=AX.X)
        nc.vector.tensor_scalar_mul(rowst[:], rowst[:], -1.0)
        t24 = sp.tile([128, NSEG], f32, tag="t24", name="t24")
        nc.vector.tensor_reduce(t24[:], m3[:].rearrange("p (s e) -> p s e", e=8),
                                axis=AX.X, op=OP.min)
        t12 = sp.tile([128, NSEG], f32, tag="t12", name="t12")
        m2v = m2[:].rearrange("p (s e) -> p s e", e=8)
        nc.vector.tensor_copy(t12[:], m2v[:, :, 3])

        e1 = sp.tile([128, NSEG * 48], f32, tag="e1", name="e1")
        p1 = sp.tile([128, NSEG * 48], f32, tag="p1", name="p1")
        Z1 = sp.tile([128, NSEG], f32, tag="Z1", name="Z1")
        for s in range(NSEG):
            nc.scalar.activation(seg(e1, s), seg(A, s), AF.Exp,
                                 bias=rowst[:, s:s + 1], scale=1.0)
            nc.vector.scalar_tensor_tensor(
                out=seg(p1, s), in0=seg(A, s), scalar=t24[:, s:s + 1],
                in1=seg(e1, s), op0=OP.is_ge, op1=OP.mult,
                accum_out=Z1[:, s:s + 1])
        r1 = sp.tile([128, NSEG], f32, tag="r1", name="r1")
        nc.vector.reciprocal(r1[:], Z1[:])
        e2 = sp.tile([128, NSEG * 48], f32, tag="e2", name="e2")
        p2 = sp.tile([128, NSEG * 48], f32, tag="p2", name="p2")
        Z2 = sp.tile([128, NSEG], f32, tag="Z2", name="Z2")
        for s in range(NSEG):
            nc.scalar.activation(seg(e2, s), seg(p1, s), AF.Exp,
                                 bias=0.0, scale=r1[:, s:s + 1])
            nc.vector.scalar_tensor_tensor(
                out=seg(p2, s), in0=seg(A, s), scalar=t12[:, s:s + 1],
                in1=seg(e2, s), op0=OP.is_ge, op1=OP.mult,
                accum_out=Z2[:, s:s + 1])
        r2 = sp.tile([128, NSEG], f32, tag="r2", name="r2")
        nc.vector.reciprocal(r2[:], Z2[:])
        r1p = sp.tile([128, NSEG], f32, tag="r1p", name="r1p")
        nc.vector.tensor_scalar_mul(r1p[:], r1[:], attns_bc[:, 0:1])
        r2p = sp.tile([128, NSEG], f32, tag="r2p", name="r2p")
        nc.vector.tensor_scalar_mul(r2p[:], r2[:], attns_bc[:, 1:2])

        ac = sp.tile([128, NSEG * 48], f32, tag="ac", name="ac")
        tmpc = sp.tile([128, NSEG * 48], f32, tag="tmpc", name="tmpc")
        for s in range(NSEG):
            nc.vector.tensor_scalar_mul(seg(tmpc, s), seg(p2, s), r2p[:, s:s + 1])
            nc.vector.scalar_tensor_tensor(
                out=seg(ac, s), in0=seg(p1, s), scalar=r1p[:, s:s + 1],
                in1=seg(tmpc, s), op0=OP.mult, op1=OP.add)
        acb = sp.tile([128, NSEG * 48], bf16, tag="acb", name="acb")
        nc.vector.tensor_copy(acb[:], ac[:])

        # ---- assemble A_cs block-diag lhsT tiles (same-partition copies) ----
        A_cs = [sp.tile([128, DIM], bf16, tag=f"Acs{ct}", name=f"Acs{ct}")
                for ct in range(3)]
        for ct in range(3):
            nc.vector.memset(A_cs[ct][:], 0.0)
        # A_cs[ct][p, C*h+d] = acb[p, 48*ct+d] where head (128*ct+p)//C == h,
        # else 0 -- masked full-partition copies (engine APs must start at
        # partition 0/32/64/96, so per-head partition slices are not usable).
        for ct in range(3):
            h0 = (128 * ct) // C
            h1 = (128 * ct + 127) // C
            for h in range(h0, h1 + 1):
                nc.vector.tensor_tensor(
                    out=A_cs[ct][:, C * h:C * (h + 1)],
                    in0=acb[:, 48 * ct:48 * (ct + 1)],
                    in1=mask3[:, ct, C * h:C * (h + 1)],
                    op=OP.mult)

        # ---- MT = A^T Wproj^T  ([s,o], fp8) ----
        mt8 = sp.tile([128, 3, DIM], fp8, tag="mt8", name="mt8")
        for st in range(3):
            psmt = ps_pool.tile([128, DIM], f32, tag=f"dw{st % 2}", name="mtps")
            for ct in range(3):
                nc.tensor.matmul(psmt[:], A_cs[ct][:, 128 * st:128 * st + 128],
                                 wprojT[ct][:], start=(ct == 0), stop=(ct == 2))
            evac(mt8[:, st, :], psmt[:], S_MT)

        # ---- out0 = MT^T V (fp8 DR) ----
        out0f8 = pp.tile([128, 3, NL], fp8, tag="u0", name=f"out0{b}")
        for mo in range(3):
            for ck in range(NCHUNK):
                pso = ps_pool.tile([128, CH], f32, tag=f"dw{ck % 4}", name="avps")
                nc.tensor.matmul(pso[:], mt8[:, 0:2, 128 * mo:128 * mo + 128],
                                 vcm8[b][:, 0:2, ck * CH:(ck + 1) * CH],
                                 perf_mode=DR, start=True, stop=False)
                nc.tensor.matmul(pso[:], mt8[:, 2, 128 * mo:128 * mo + 128],
                                 vcm8[b][:, 2, ck * CH:(ck + 1) * CH],
                                 start=False, stop=True)
                evac(out0f8[:, mo, ck * CH:(ck + 1) * CH], pso[:], EV_O)

        # ---- prompt branches; activations grouped by function ----
        g16a = pp.tile([128, 6, NL], bf16, tag="spm", name=f"g16{b}")
        for bi, br in enumerate(("chr", "detg")):   # all GELUs together
            for mo in range(3):
                for ck in range(NCHUNK):
                    psg = ps_pool.tile([128, CH], f32, tag=f"dw{ck % 4}", name="gps")
                    nc.tensor.matmul(psg[:], w1T8[br][:, 0:2, 128 * mo:128 * mo + 128],
                                     out0f8[:, 0:2, ck * CH:(ck + 1) * CH],
                                     perf_mode=DR, start=True, stop=False)
                    nc.tensor.matmul(psg[:], w1T8[br][:, 2, 128 * mo:128 * mo + 128],
                                     out0f8[:, 2, ck * CH:(ck + 1) * CH],
                                     start=False, stop=True)
                    nc.scalar.activation(g16a[:, 3 * bi + mo, ck * CH:(ck + 1) * CH],
                                         psg[:], AF.Gelu,
                                         bias=b1[br][:, mo:mo + 1], scale=EV_G)
        gate16 = {}
        for bi, br in enumerate(("chr", "detg")):   # all sigmoids together
            gate16[br] = sp.tile([1, NL], bf16, tag=f"gate{bi}", name=f"gate{br}")
            for ck in range(NCHUNK):
                psgt = ps_pool.tile([16, CH], f32, tag=f"dw{ck % 4}", name="gateps")
                for kt in range(3):
                    nc.tensor.matmul(psgt[:], w2p[br][:, kt, :],
                                     g16a[:, 3 * bi + kt, ck * CH:(ck + 1) * CH],
                                     start=(kt == 0), stop=(kt == 2))
                nc.scalar.activation(gate16[br][:, ck * CH:(ck + 1) * CH], psgt[0:1, :],
                                     AF.Sigmoid, bias=b2sb[0:1, bi:bi + 1], scale=1.0)
        gated16 = {}
        for bi, br in enumerate(("chr", "detg")):
            gated16[br] = sp.tile([64, NL], bf16, tag=f"gtd{bi}", name=f"gated{br}")
            for ck in range(NCHUNK):
                psgb = ps_pool.tile([64, CH], f32, tag=f"dw{(ck + 2) % 4}", name="gbps")
                nc.tensor.matmul(psgb[:], ones1b[:], gate16[br][0:1, ck * CH:(ck + 1) * CH],
                                 start=True, stop=True)
                nc.vector.scalar_tensor_tensor(
                    out=gated16[br][:, ck * CH:(ck + 1) * CH],
                    in0=gk_sb[(b, br)][:, ck * CH:(ck + 1) * CH],
                    scalar=1.0, in1=psgb[:], op0=OP.mult, op1=OP.mult)
        # ---- alpha broadcast (Identity only -- no act-table load) ----
        al16 = pp.tile([128, NL], bf16, tag="u7", name="al16")
        oma16 = pp.tile([128, NL], bf16, tag="bigA", name="oma16")
        for ck in range(NCHUNK):
            psal = ps_pool.tile([128, CH], f32, tag=f"dw{ck % 4}", name="alps")
            nc.tensor.matmul(psal[:], zrep16[:], gk_sb[(b, "detg")][:, ck * CH:(ck + 1) * CH],
                             start=True, stop=True)
            nc.scalar.copy(al16[:, ck * CH:(ck + 1) * CH], psal[:])
            nc.scalar.activation(oma16[:, ck * CH:(ck + 1) * CH], psal[:],
                                 AF.Identity, bias=1.0, scale=-1.0)

        # ---- wt conv + silu, blended per mo so the tail overlaps ----
        for mo in range(3):
            pr = {}
            for bi, br in enumerate(("chr", "detg")):
                pr[br] = pp.tile([128, NL], bf16, tag=f"u{1 + 2 * bi + (mo % 2)}",
                                 name=f"pr{br}")
                for ck in range(NCHUNK):
                    pst2 = ps_pool.tile([128, CH], f32, tag=f"dw{ck % 4}", name="transps")
                    nc.tensor.matmul(pst2[:], wtT[br][:, 128 * mo:128 * mo + 128],
                                     gated16[br][:, ck * CH:(ck + 1) * CH],
                                     start=True, stop=True)
                    nc.scalar.activation(pr[br][:, ck * CH:(ck + 1) * CH], pst2[:],
                                         AF.Silu, bias=bt[br][:, mo:mo + 1], scale=1.0)
            pa = pp.tile([128, NL], bf16, tag="u5", name="pa")
            nc.vector.tensor_mul(pa[:], pr["chr"][:], oma16[:])
            pb = pp.tile([128, NL], bf16, tag="u6", name="pb")
            # keep batch 0 off the gpsimd queue -- its chain would otherwise
            # wait behind the batch-1 AllReduce queued on the same engine
            if b == 0:
                nc.vector.tensor_mul(pb[:], pr["detg"][:], al16[:])
            else:
                nc.gpsimd.tensor_mul(pb[:], pr["detg"][:], al16[:])
            p12 = sp.tile([128, NL], bf16, tag="p12", name="p12")
            nc.vector.tensor_add(p12[:], pa[:], pb[:])
            fin = pp.tile([128, NL], f32, tag="spm", name="fin")
            nc.vector.scalar_tensor_tensor(
                out=fin[:], in0=out0f8[:, mo, :], scalar=1.0 / S_O, in1=p12[:],
                op0=OP.mult, op1=OP.add)
            nc.sync.dma_start(t["OUT"][b, mo], fin[:])


_PROG = None


def _program():
    global _PROG
    if _PROG is None:
        _PROG = build_program()
    return _PROG


def kernel(**inputs):
    from concourse.bass_utils import run_bass_kernel_spmd
    nc = _program()
    consts = prep_constants(inputs)
    maps = shard_inputs(inputs, consts)
    res = run_bass_kernel_spmd(nc, maps, list(range(NCORE)))
    out = np.empty((B, DIM, Himg, Wimg), np.float32)
    for ci in range(NCORE):
        o = res.results[ci]["OUT"].reshape(B, DIM, ROWS, Wimg)
        out[:, :, ROWS * ci:ROWS * (ci + 1), :] = o
    return out


# revision 27
# speedup vs baseline: 1.0392x; 1.0392x over previous
"""TRN2 Bass kernel for nn_CrossAttention (sparse channel attention + prompt
fusion), sharded spatially over 8 NeuronCores (16 image rows + halo per core).

v2 design:
- fp8(e4m3) DoubleRow matmuls (2 K-subtiles per instruction) for the qkv 1x1
  conv, the 3x3 depthwise conv (as K=128 diagonal matmuls with vertically
  paired taps), attn@v and the prompt w1 conv.  Static power-of-2 scales keep
  everything out of the fp8 subnormal range; scales are divided out in the
  PSUM evacuations.
- proj is folded into attn@v:  out0 = (Wproj @ A_blockdiag) @ V, with
  MT = A^T Wproj^T computed once per batch on the PE (A assembled from the
  softmax output by same-partition block copies -- no transposes needed).
- the per-(batch) Gram AllReduces are split so each overlaps the other
  batch's compute.
"""
import sys

for _p in ("/opt/trn_rl_repo", "/root/.axon_site/_ro/trn_rl_repo"):
    if _p not in sys.path:
        sys.path.insert(0, _p)

import numpy as np

B, DIM, HEADS, Himg, Wimg = 2, 384, 8, 128, 128
C = DIM // HEADS            # 48
QKVC = 3 * DIM              # 1152
NCORE = 8
ROWS = Himg // NCORE        # 16 rows per core
NL = ROWS * Wimg            # 2048 local pixels
HR = ROWS + 2               # 18 rows with halo
NH = HR * Wimg              # 2304 halo pixels
PADW = 144                  # padded row pitch (16-aligned for DoubleRow)
NPAD = HR * PADW            # 2592
NT = 11                     # qkv out tiles: q 0-3 (head-padded), k 4-7, v 8-10
NSEG = 3                    # 384 attention rows per batch / 128
CH = 512
NCHUNK = NL // CH           # 4

# power-of-2 scales (see fp8_sim2.py numerics check)
S_X, S_WQ, S_QKV, S_WD = 16.0, 512.0, 32.0, 512.0
S_V, S_MT, S_O, S_W1 = 1024.0, 512.0, 256.0, 512.0
EV_QKV = S_QKV / (S_X * S_WQ)     # qkv psum -> fp8 qkvpad
EV_V = S_V / (S_QKV * S_WD)       # dw psum -> fp8 v
EV_QK = 1024.0 / (S_QKV * S_WD)   # dw psum -> bf16 q,k (scale cancels in norm)
EV_O = S_O / (S_MT * S_V)         # attn@v psum -> fp8 out0
EV_G = 1.0 / (S_O * S_W1)         # w1 psum -> true scale (gelu input)


def qkv_out_map():
    """raw qkv channel -> (tile, partition) in the head-padded layout."""
    perm = np.zeros((QKVC, 2), np.int64)
    for c in range(DIM):            # q
        h, j = c // C, c % C
        perm[c] = (h // 2, 64 * (h % 2) + j)
    for c in range(DIM):            # k
        h, j = c // C, c % C
        perm[DIM + c] = (4 + h // 2, 64 * (h % 2) + j)
    for j in range(DIM):            # v
        perm[2 * DIM + j] = (8 + j // 128, j % 128)
    return perm


def prep_constants(inputs):
    import ml_dtypes
    bf16 = ml_dtypes.bfloat16
    fp8 = ml_dtypes.float8_e4m3fn
    perm = qkv_out_map()
    w_qkv = np.asarray(inputs["w_qkv"], np.float32)
    w_dw = np.asarray(inputs["w_dw"], np.float32).reshape(QKVC, 9)
    w_proj = np.asarray(inputs["w_proj"], np.float32)

    # wqkvT8 [128, 3, NT*128]: permuted output channels, scaled
    wqkvT8 = np.zeros((128, 3, NT * 128), np.float32)
    for rc in range(QKVC):
        m, p = perm[rc]
        wqkvT8[:, :, m * 128 + p] = w_qkv[rc].reshape(3, 128).T * S_WQ
    # dwdiag8 [128, NT*9*128]: per (tile m, slot q) a diagonal block.
    # Slot order TAPORD puts the vertical DoubleRow pairs (dy=0,dy=1) at
    # adjacent slots 2*dx, 2*dx+1; the dy=2 singles at slots 6,7,8.
    TAPORD = [0, 3, 1, 4, 2, 5, 6, 7, 8]
    dwdiag8 = np.zeros((128, NT * 9 * 128), np.float32)
    for rc in range(QKVC):
        m, p = perm[rc]
        for q, tp in enumerate(TAPORD):
            dwdiag8[p, (m * 9 + q) * 128 + p] = w_dw[rc, tp] * S_WD

    def lhsT3(w):   # [out, 384] -> [3, 128, out]
        return np.transpose(np.asarray(w, np.float32).reshape(-1, 3, 128),
                            (1, 2, 0)).copy()

    def lhsT3f(w, s):  # [out, 384] -> [128, 3, out] scaled
        return np.ascontiguousarray(
            np.transpose(np.asarray(w, np.float32).reshape(-1, 3, 128),
                         (2, 1, 0)) * s)

    w2pad = {}
    for br, key in (("chr", "chr_w2"), ("detg", "detg_w2")):
        w2 = np.asarray(inputs[key], np.float32)       # [1, 384]
        pad = np.zeros((128, 3, 16), np.float32)
        pad[:, :, 0] = w2.reshape(3, 128).T
        w2pad[br] = pad

    sel = np.zeros((HEADS, NSEG * 128), np.float32)
    for s in range(NSEG):
        for p in range(128):
            sel[(128 * s + p) // C, s * 128 + p] = 1.0

    # block-diag assembly mask: mask3[p, ct, C*h+d] = ((128*ct+p)//C == h)
    mask3 = np.zeros((128, 3, DIM), np.float32)
    for ct in range(3):
        for p in range(128):
            h = (128 * ct + p) // C
            mask3[p, ct, C * h:C * (h + 1)] = 1.0

    out = dict(
        wqkvT8=wqkvT8.astype(fp8),
        dwdiag8=dwdiag8.astype(fp8),
        wprojT=lhsT3(w_proj).astype(bf16),                       # [3,128,384]
        w1T8_chr=lhsT3f(inputs["chr_w1"], S_W1).astype(fp8),     # [128,3,384]
        w1T8_detg=lhsT3f(inputs["detg_w1"], S_W1).astype(fp8),
        w2p_chr=w2pad["chr"].astype(bf16),                       # [128,3,16]
        w2p_detg=w2pad["detg"].astype(bf16),
        wtT_chr=np.asarray(inputs["chr_wt"], np.float32).T.copy().astype(bf16),
        wtT_detg=np.asarray(inputs["detg_wt"], np.float32).T.copy().astype(bf16),
        b1_chr=np.ascontiguousarray(np.asarray(inputs["chr_b1"], np.float32).reshape(3, 128).T),
        b1_detg=np.ascontiguousarray(np.asarray(inputs["detg_b1"], np.float32).reshape(3, 128).T),
        bt_chr=np.ascontiguousarray(np.asarray(inputs["chr_bt"], np.float32).reshape(3, 128).T),
        bt_detg=np.ascontiguousarray(np.asarray(inputs["detg_bt"], np.float32).reshape(3, 128).T),
        b2s=np.asarray([float(np.asarray(inputs["chr_b2"]).ravel()[0]),
                        float(np.asarray(inputs["detg_b2"]).ravel()[0])],
                       np.float32).reshape(1, 2),
        temp8=np.asarray(inputs["temperature"], np.float32).reshape(HEADS, 1).copy(),
        attns2=np.asarray(inputs["attns"], np.float32).reshape(1, 2).copy(),
        detg_z=np.asarray(inputs["detg_z"], np.float32).reshape(1, 64).copy(),
        sel3=sel,
        mask3=mask3.astype(bf16),
        ones1f=np.ones((1, 128), np.float32),
        ones64f=np.ones((64, 128), np.float32),
        ones1b=np.ones((1, 64), np.float32).astype(bf16),
    )
    return out


def shard_inputs(inputs, consts):
    import ml_dtypes
    bf16 = ml_dtypes.bfloat16
    fp8 = ml_dtypes.float8_e4m3fn
    x = np.asarray(inputs["x"], np.float32)
    gk0 = np.asarray(inputs["gk0"], np.float32)
    gk1 = np.asarray(inputs["gk1"], np.float32)
    xp = np.pad(x, ((0, 0), (0, 0), (1, 1), (0, 0)))
    maps = []
    for ci in range(NCORE):
        r0 = ROWS * ci
        xs = xp[:, :, r0:r0 + HR, :].reshape(B, 3, 128, NH) * S_X
        g0 = gk0[:, :, r0:r0 + ROWS, :].reshape(B, 64, NL)
        g1 = gk1[:, :, r0:r0 + ROWS, :].reshape(B, 64, NL)
        m = {"x_s": np.ascontiguousarray(xs).astype(fp8),
             "gk0_s": np.ascontiguousarray(g0).astype(bf16),
             "gk1_s": np.ascontiguousarray(g1).astype(bf16)}
        m.update(consts)
        maps.append(m)
    return maps


from contextlib import ExitStack

import concourse.bass as bass
import concourse.tile as tile
import concourse.mybir as mybir
from concourse import bacc

f32 = mybir.dt.float32
bf16 = mybir.dt.bfloat16
fp8 = mybir.dt.float8e4
AX = mybir.AxisListType
OP = mybir.AluOpType
AF = mybir.ActivationFunctionType
DR = mybir.MatmulPerfMode.DoubleRow


def build_program():
    nc = bacc.Bacc("TRN2", debug=False, num_devices=NCORE,
                   target_bir_lowering=False)

    def din(name, shape, dt):
        return nc.dram_tensor(name, list(shape), dt, kind="ExternalInput").ap()

    t = {}
    t["x_s"] = din("x_s", (B, 3, 128, NH), fp8)
    t["gk0_s"] = din("gk0_s", (B, 64, NL), bf16)
    t["gk1_s"] = din("gk1_s", (B, 64, NL), bf16)
    t["wqkvT8"] = din("wqkvT8", (128, 3, NT * 128), fp8)
    t["dwdiag8"] = din("dwdiag8", (128, NT * 9 * 128), fp8)
    t["wprojT"] = din("wprojT", (3, 128, DIM), bf16)
    for br in ("chr", "detg"):
        t[f"w1T8_{br}"] = din(f"w1T8_{br}", (128, 3, DIM), fp8)
        t[f"w2p_{br}"] = din(f"w2p_{br}", (128, 3, 16), bf16)
        t[f"wtT_{br}"] = din(f"wtT_{br}", (64, DIM), bf16)
        t[f"b1_{br}"] = din(f"b1_{br}", (128, 3), f32)
        t[f"bt_{br}"] = din(f"bt_{br}", (128, 3), f32)
    t["b2s"] = din("b2s", (1, 2), f32)
    t["temp8"] = din("temp8", (HEADS, 1), f32)
    t["attns2"] = din("attns2", (1, 2), f32)
    t["detg_z"] = din("detg_z", (1, 64), f32)
    t["sel3"] = din("sel3", (HEADS, NSEG * 128), f32)
    t["mask3"] = din("mask3", (128, 3, DIM), bf16)
    t["ones1f"] = din("ones1f", (1, 128), f32)
    t["ones64f"] = din("ones64f", (64, 128), f32)
    t["ones1b"] = din("ones1b", (1, 64), bf16)

    t["OUT"] = nc.dram_tensor("OUT", [B, 3, 128, NL], f32, kind="ExternalOutput").ap()

    for b in range(B):
        t[f"g3_part{b}"] = nc.dram_tensor(f"g3_part{b}", [HEADS, 96, 96], f32)
        t[f"g3_all{b}"] = nc.dram_tensor(f"g3_all{b}", [HEADS, 96, 96], f32,
                                         addr_space="Shared")
        t[f"qk_dense{b}"] = nc.dram_tensor(f"qk_dense{b}", [NSEG * 128, C], f32)
        t[f"rq_flat{b}"] = nc.dram_tensor(f"rq_flat{b}", [NSEG * 128], f32)
    t["zb"] = nc.dram_tensor("zb", [64], f32)

    with tile.TileContext(nc) as tc, ExitStack() as ctx:
        _body(tc, ctx, t)
    nc.compile()
    return nc


def _body(tc, ctx, t):
    nc = tc.nc
    wp = ctx.enter_context(tc.tile_pool(name="wp", bufs=1))
    pp = ctx.enter_context(tc.tile_pool(name="pp", bufs=1))
    sp = ctx.enter_context(tc.tile_pool(name="sp", bufs=1))
    ps_pool = ctx.enter_context(tc.tile_pool(name="ps", bufs=2, space="PSUM"))

    ec = [0]

    def evac(out_ap, in_ap, scale):
        """PSUM evacuation with scale, alternating scalar/vector."""
        if ec[0] % 3 == 0:
            nc.scalar.activation(out_ap, in_ap, AF.Identity, bias=0.0, scale=scale)
        else:
            nc.vector.tensor_scalar_mul(out_ap, in_ap, scale)
        ec[0] += 1

    # ---------------- constants into SBUF ----------------
    # x first -- the first qkv matmul needs only x and wqv
    x_sb = [wp.tile([128, 3, NH], fp8, tag=f"x{b}", name=f"x{b}") for b in range(B)]
    for b in range(B):
        nc.sync.dma_start(x_sb[b][:], t["x_s"][b].rearrange("k p s -> p k s"))
    wqv = wp.tile([128, 3, NT * 128], fp8, tag="wqv", name="wqv")
    nc.sync.dma_start(wqv[:], t["wqkvT8"])
    dwd = wp.tile([128, NT * 9 * 128], fp8, tag="dwd", name="dwd")
    nc.sync.dma_start(dwd[:], t["dwdiag8"])
    wprojT = [wp.tile([128, DIM], bf16, tag=f"wproj{k}", name=f"wproj{k}") for k in range(3)]
    for k in range(3):
        nc.sync.dma_start(wprojT[k][:], t["wprojT"][k])
    w1T8, w2p, wtT, b1, bt = {}, {}, {}, {}, {}
    for br in ("chr", "detg"):
        w1T8[br] = wp.tile([128, 3, DIM], fp8, tag=f"w1{br}", name=f"w1{br}")
        nc.sync.dma_start(w1T8[br][:], t[f"w1T8_{br}"])
        w2p[br] = wp.tile([128, 3, 16], bf16, tag=f"w2{br}", name=f"w2{br}")
        nc.sync.dma_start(w2p[br][:], t[f"w2p_{br}"])
        wtT[br] = wp.tile([64, DIM], bf16, tag=f"wt{br}", name=f"wt{br}")
        nc.sync.dma_start(wtT[br][:], t[f"wtT_{br}"])
        b1[br] = wp.tile([128, 3], f32, tag=f"b1{br}", name=f"b1{br}")
        nc.sync.dma_start(b1[br][:], t[f"b1_{br}"])
        bt[br] = wp.tile([128, 3], f32, tag=f"bt{br}", name=f"bt{br}")
        nc.sync.dma_start(bt[br][:], t[f"bt_{br}"])
    b2sb = wp.tile([1, 2], f32, tag="b2", name="b2")
    nc.sync.dma_start(b2sb[:], t["b2s"])
    tempc = wp.tile([HEADS, 1], f32, tag="temp", name="temp")
    nc.sync.dma_start(tempc[:], t["temp8"])
    attns_sb = wp.tile([1, 2], f32, tag="attns", name="attns")
    nc.sync.dma_start(attns_sb[:], t["attns2"])
    zrow = wp.tile([1, 64], f32, tag="zrow", name="zrow")
    nc.sync.dma_start(zrow[:], t["detg_z"])
    sel3 = wp.tile([HEADS, NSEG * 128], f32, tag="sel3", name="sel3")
    nc.sync.dma_start(sel3[:], t["sel3"])
    mask3 = wp.tile([128, 3, DIM], bf16, tag="mask3", name="mask3")
    nc.sync.dma_start(mask3[:], t["mask3"])
    ones1f = wp.tile([1, 128], f32, tag="ones1f", name="ones1f")
    nc.sync.dma_start(ones1f[:], t["ones1f"])
    ones64f = wp.tile([64, 128], f32, tag="ones64f", name="ones64f")
    nc.sync.dma_start(ones64f[:], t["ones64f"])
    ones1b = wp.tile([1, 64], bf16, tag="ones1b", name="ones1b")
    nc.sync.dma_start(ones1b[:], t["ones1b"])
    gk_sb = {}
    for b in range(B):
        for br, gk_d in (("chr", t["gk0_s"]), ("detg", t["gk1_s"])):
            g = wp.tile([64, NL], bf16, tag=f"gk{b}{br}", name=f"gk{b}{br}")
            nc.sync.dma_start(g[:], gk_d[b])
            gk_sb[(b, br)] = g

    # per-batch persistent: v (fp8, channel-major) tiles
    vcm8 = [wp.tile([128, 3, NL], fp8, tag=f"v{b}", name=f"v{b}") for b in range(B)]
    dwdv = dwd[:].rearrange("p (m t c) -> p m t c", t=9, c=128)

    # ================= per-batch compute: qkv, dw, gram =================
    for b in range(B):
        qkvpad = pp.tile([128, NT * NPAD], fp8, tag="bigA", name=f"qkvpad{b}")
        pvm = qkvpad[:].rearrange("p (m s) -> p m s", s=NPAD)
        xv = x_sb[b][:]
        qk_cm = [pp.tile([128, NL], bf16, tag=f"u{m}", name=f"qk{b}_{m}")
                 for m in range(8)]

        def qkv_tile(m):
            # lhsT kept constant across chunk groups (LDWEIGHTS amortization)
            pvr = pvm[:, m, :].rearrange("p (r w) -> p r w", w=PADW)
            nc.vector.memset(pvr[:, :, 0:1], 0.0)
            nc.vector.memset(pvr[:, :, 129:130], 0.0)
            for g0, g1 in ((0, 4), (4, 6)):      # chunk groups of <=4
                psqs = [ps_pool.tile([128, 384], f32, tag=f"dw{i % 4}", name="qkvps")
                        for i in range(g0, g1)]
                for i, nck in enumerate(range(g0, g1)):
                    nc.tensor.matmul(psqs[i][:], wqv[:, 0:2, m * 128:(m + 1) * 128],
                                     xv[:, 0:2, nck * 384:(nck + 1) * 384],
                                     perf_mode=DR, start=True, stop=False)
                for i, nck in enumerate(range(g0, g1)):
                    nc.tensor.matmul(psqs[i][:], wqv[:, 2, m * 128:(m + 1) * 128],
                                     xv[:, 2, nck * 384:(nck + 1) * 384],
                                     start=False, stop=True)
                for i, nck in enumerate(range(g0, g1)):
                    evac(pvr[:, 3 * nck:3 * nck + 3, 1:129],
                         psqs[i][:].rearrange("p (r w) -> p r w", w=128), EV_QKV)

        def dw_tile(m):
            # taps outer, rows inner: lhsT constant across the 16 row matmuls
            pvr = pvm[:, m, :].rearrange("p (r w) -> p r w", w=PADW)
            psd = [ps_pool.tile([128, 512], f32, tag=f"dw{yg}", name="dwps")
                   for yg in range(4)]
            for dx in range(3):                  # DR pairs (dy=0,1)
                for y in range(16):
                    out = psd[y // 4][:, 128 * (y % 4):128 * (y % 4) + 128]
                    nc.tensor.matmul(
                        out, dwdv[:, m, 2 * dx:2 * dx + 2, :],
                        pvr[:, y:y + 2, dx:dx + 128],
                        perf_mode=DR, start=(dx == 0), stop=False,
                        skip_group_check=True)
            for dx in range(3):                  # dy=2 singles
                for y in range(16):
                    out = psd[y // 4][:, 128 * (y % 4):128 * (y % 4) + 128]
                    nc.tensor.matmul(
                        out, dwdv[:, m, 6 + dx, :],
                        pvr[:, y + 2, dx:dx + 128],
                        start=False, stop=(dx == 2),
                        skip_group_check=True)
            for yg in range(4):
                if m < 8:
                    evac(qk_cm[m][:, 512 * yg:512 * (yg + 1)], psd[yg][:], EV_QK)
                else:
                    evac(vcm8[b][:, m - 8, 512 * yg:512 * (yg + 1)], psd[yg][:], EV_V)

        # q,k tiles first so the Gram AllReduce can start before the v work
        for m in range(8):
            qkv_tile(m)
        for m in range(8):
            dw_tile(m)

        # ---- transpose q,k to pixel-major, stacked per head ----
        s_pm = pp.tile([128, 16 * 768], bf16, tag="spm", name=f"spm{b}")
        spm3 = s_pm[:].rearrange("p (c blk) -> p c blk", blk=768)
        for h in range(HEADS):
            qb = 64 * (h % 2)
            nc.sync.dma_start_transpose(
                spm3[:, :, 96 * h:96 * h + 48], qk_cm[h // 2][qb:qb + 48, :])
            nc.sync.dma_start_transpose(
                spm3[:, :, 96 * h + 48:96 * h + 96], qk_cm[4 + h // 2][qb:qb + 48, :])

        # ---- stacked Gram [96,96] per head ----
        g3sb = pp.tile([96, 8 * 96], f32, tag="g3sb", name=f"g3sb{b}")
        for h in range(HEADS):
            psg = ps_pool.tile([96, 96], f32, tag=f"dw{h % 4}", name="g3ps")
            for ckk in range(16):
                lhs = spm3[:, ckk, 96 * h:96 * h + 96]
                nc.tensor.matmul(psg[:], lhs, lhs,
                                 start=(ckk == 0), stop=(ckk == 15))
            nc.vector.tensor_copy(g3sb[:, 96 * h:96 * (h + 1)], psg[:])
        nc.sync.dma_start(
            t[f"g3_part{b}"].ap().rearrange("h r c -> r h c"),
            g3sb[:].rearrange("r (h c) -> r h c", c=96))

        nc.gpsimd.collective_compute(
            "AllReduce", OP.add, replica_groups=[list(range(NCORE))],
            ins=[t[f"g3_part{b}"].ap().opt()],
            outs=[t[f"g3_all{b}"].ap().opt()])

        # v tiles after the AllReduce is queued
        for m in range(8, NT):
            qkv_tile(m)
        for m in range(8, NT):
            dw_tile(m)

    # attns broadcast [128, 2]
    ps_a = ps_pool.tile([128, 2], f32, tag="dw0", name="attnsps")
    nc.tensor.matmul(ps_a[:], ones1f[:], attns_sb[:], start=True, stop=True)
    attns_bc = wp.tile([128, 2], f32, tag="attnsbc", name="attnsbc")
    nc.vector.tensor_copy(attns_bc[:], ps_a[:])

    # z-bar [64, 128] bf16 (for alpha)
    zsq = sp.tile([1, 64], f32, tag="zsq", name="zsq")
    nc.scalar.square(zsq[:], zrow[:])
    zss = sp.tile([1, 1], f32, tag="zss", name="zss")
    nc.vector.reduce_sum(zss[:], zsq[:], axis=AX.X)
    nc.scalar.sqrt(zss[:], zss[:])
    nc.vector.tensor_scalar_max(zss[:], zss[:], 1e-12)
    zrs = sp.tile([1, 1], f32, tag="zrs", name="zrs")
    nc.vector.reciprocal(zrs[:], zss[:])
    zn = sp.tile([1, 64], f32, tag="zn", name="zn")
    nc.vector.tensor_scalar_mul(zn[:], zrow[:], zrs[:, 0:1])
    nc.sync.dma_start(t["zb"].ap().rearrange("(a b) -> a b", a=1), zn[:])
    zcol = sp.tile([64, 1], f32, tag="zcol", name="zcol")
    nc.sync.dma_start(zcol[:], t["zb"].ap().rearrange("(p a) -> p a", a=1))
    zrep = sp.tile([64, 128], f32, tag="zrep", name="zrep")
    nc.vector.tensor_scalar_mul(zrep[:], ones64f[:], zcol[:, 0:1])
    zrep16 = wp.tile([64, 128], bf16, tag="zrep16", name="zrep16")
    nc.vector.tensor_copy(zrep16[:], zrep[:])


    # ================= per-batch attention matrix + output chain ========
    for b in range(B):
        g3a = t[f"g3_all{b}"]
        # norms from the stacked-Gram diagonals
        norm2 = sp.tile([HEADS, 96], f32, tag="norm2", name="norm2")
        nc.sync.dma_start(norm2[:, 0:48],
                          bass.AP(tensor=g3a, offset=0, ap=[[96 * 96, HEADS], [97, 48]]))
        nc.sync.dma_start(norm2[:, 48:96],
                          bass.AP(tensor=g3a, offset=48 * 96 + 48,
                                  ap=[[96 * 96, HEADS], [97, 48]]))
        nc.scalar.sqrt(norm2[:], norm2[:])
        nc.vector.tensor_scalar_max(norm2[:], norm2[:], 1e-12)
        rn = sp.tile([HEADS, 96], f32, tag="rn", name="rn")
        nc.vector.reciprocal(rn[:], norm2[:])
        rqf = sp.tile([HEADS, 48], f32, tag="rqf", name="rqf")
        nc.vector.tensor_scalar_mul(rqf[:], rn[:, 0:48], tempc[:, 0:1])
        nc.sync.dma_start(t[f"rq_flat{b}"].ap().rearrange("(a c) -> a c", a=HEADS), rqf[:])
        rq_seg = sp.tile([128, NSEG], f32, tag="rqseg", name="rqseg")
        nc.sync.dma_start(rq_seg[:],
                          t[f"rq_flat{b}"].ap().rearrange("(s p) -> p s", s=NSEG))
        psrk = ps_pool.tile([128, NSEG * 48], f32, tag="dw0", name="rkps")
        for s in range(NSEG):
            nc.tensor.matmul(psrk[:, 48 * s:48 * s + 48],
                             sel3[:, 128 * s:128 * s + 128], rn[:, 48:96],
                             start=True, stop=True)
        rk_bc = sp.tile([128, NSEG * 48], f32, tag="rkbc", name="rkbc")
        nc.vector.tensor_copy(rk_bc[:], psrk[:])
        nc.sync.dma_start(
            t[f"qk_dense{b}"].ap().rearrange("(h c) d -> h c d", h=HEADS),
            g3a.ap()[:, 0:48, 48:96])
        G_seg = sp.tile([128, NSEG * 48], f32, tag="gseg", name="gseg")
        nc.sync.dma_start(G_seg[:].rearrange("p (s d) -> p s d", s=NSEG),
                          t[f"qk_dense{b}"].ap().rearrange("(s p) d -> p s d", s=NSEG))

        A = sp.tile([128, NSEG * 48], f32, tag="A", name="A")
        seg = lambda tl, s: tl[:, 48 * s:48 * s + 48]
        for s in range(NSEG):
            nc.vector.scalar_tensor_tensor(
                out=seg(A, s), in0=seg(G_seg, s), scalar=rq_seg[:, s:s + 1],
                in1=seg(rk_bc, s), op0=OP.mult, op1=OP.mult)

        m1 = sp.tile([128, NSEG * 8], f32, tag="m1", name="m1")
        m2 = sp.tile([128, NSEG * 8], f32, tag="m2", name="m2")
        m3 = sp.tile([128, NSEG * 8], f32, tag="m3", name="m3")
        At1 = sp.tile([128, NSEG * 48], f32, tag="At1", name="At1")
        At2 = sp.tile([128, NSEG * 48], f32, tag="At2", name="At2")
        for s in range(NSEG):
            nc.vector.max(m1[:, 8 * s:8 * s + 8], seg(A, s))
            nc.vector.match_replace(seg(At1, s), m1[:, 8 * s:8 * s + 8], seg(A, s), -1e30)
            nc.vector.max(m2[:, 8 * s:8 * s + 8], seg(At1, s))
            nc.vector.match_replace(seg(At2, s), m2[:, 8 * s:8 * s + 8], seg(At1, s), -1e30)
            nc.vector.max(m3[:, 8 * s:8 * s + 8], seg(At2, s))

        rowst = sp.tile([128, NSEG], f32, tag="rowst", name="rowst")
        nc.vector.reduce_max(rowst[:], m1[:].rearrange("p (s e) -> p s e", e=8), axis=AX.X)
        nc.vector.tensor_scalar_mul(rowst[:], rowst[:], -1.0)
        t24 = sp.tile([128, NSEG], f32, tag="t24", name="t24")
        nc.vector.tensor_reduce(t24[:], m3[:].rearrange("p (s e) -> p s e", e=8),
                                axis=AX.X, op=OP.min)
        t12 = sp.tile([128, NSEG], f32, tag="t12", name="t12")
        m2v = m2[:].rearrange("p (s e) -> p s e", e=8)
        nc.vector.tensor_copy(t12[:], m2v[:, :, 3])

        e1 = sp.tile([128, NSEG * 48], f32, tag="e1", name="e1")
        p1 = sp.tile([128, NSEG * 48], f32, tag="p1", name="p1")
        Z1 = sp.tile([128, NSEG], f32, tag="Z1", name="Z1")
        for s in range(NSEG):
            nc.scalar.activation(seg(e1, s), seg(A, s), AF.Exp,
                                 bias=rowst[:, s:s + 1], scale=1.0)
            nc.vector.scalar_tensor_tensor(
                out=seg(p1, s), in0=seg(A, s), scalar=t24[:, s:s + 1],
                in1=seg(e1, s), op0=OP.is_ge, op1=OP.mult,
                accum_out=Z1[:, s:s + 1])
        r1 = sp.tile([128, NSEG], f32, tag="r1", name="r1")
        nc.vector.reciprocal(r1[:], Z1[:])
        e2 = sp.tile([128, NSEG * 48], f32, tag="e2", name="e2")
        p2 = sp.tile([128, NSEG * 48], f32, tag="p2", name="p2")
        Z2 = sp.tile([128, NSEG], f32, tag="Z2", name="Z2")
        for s in range(NSEG):
            nc.scalar.activation(seg(e2, s), seg(p1, s), AF.Exp,
                                 bias=0.0, scale=r1[:, s:s + 1])
            nc.vector.scalar_tensor_tensor(
                out=seg(p2, s), in0=seg(A, s), scalar=t12[:, s:s + 1],
                in1=seg(e2, s), op0=OP.is_ge, op1=OP.mult,
                accum_out=Z2[:, s:s + 1])
        r2 = sp.tile([128, NSEG], f32, tag="r2", name="r2")
        nc.vector.reciprocal(r2[:], Z2[:])
        r1p = sp.tile([128, NSEG], f32, tag="r1p", name="r1p")
        nc.vector.tensor_scalar_mul(r1p[:], r1[:], attns_bc[:, 0:1])
        r2p = sp.tile([128, NSEG], f32, tag="r2p", name="r2p")
        nc.vector.tensor_scalar_mul(r2p[:], r2[:], attns_bc[:, 1:2])

        ac = sp.tile([128, NSEG * 48], f32, tag="ac", name="ac")
        tmpc = sp.tile([128, NSEG * 48], f32, tag="tmpc", name="tmpc")
        for s in range(NSEG):
            nc.vector.tensor_scalar_mul(seg(tmpc, s), seg(p2, s), r2p[:, s:s + 1])
            nc.vector.scalar_tensor_tensor(
                out=seg(ac, s), in0=seg(p1, s), scalar=r1p[:, s:s + 1],
                in1=seg(tmpc, s), op0=OP.mult, op1=OP.add)
        acb = sp.tile([128, NSEG * 48], bf16, tag="acb", name="acb")
        nc.vector.tensor_copy(acb[:], ac[:])

        # ---- assemble A_cs block-diag lhsT tiles (same-partition copies) ----
        A_cs = [sp.tile([128, DIM], bf16, tag=f"Acs{ct}", name=f"Acs{ct}")
                for ct in range(3)]
        for ct in range(3):
            nc.vector.memset(A_cs[ct][:], 0.0)
        # A_cs[ct][p, C*h+d] = acb[p, 48*ct+d] where head (128*ct+p)//C == h,
        # else 0 -- masked full-partition copies (engine APs must start at
        # partition 0/32/64/96, so per-head partition slices are not usable).
        for ct in range(3):
            h0 = (128 * ct) // C
            h1 = (128 * ct + 127) // C
            for h in range(h0, h1 + 1):
                nc.vector.tensor_tensor(
                    out=A_cs[ct][:, C * h:C * (h + 1)],
                    in0=acb[:, 48 * ct:48 * (ct + 1)],
                    in1=mask3[:, ct, C * h:C * (h + 1)],
                    op=OP.mult)

        # ---- MT = A^T Wproj^T  ([s,o], fp8) ----
        mt8 = sp.tile([128, 3, DIM], fp8, tag="mt8", name="mt8")
        for st in range(3):
            psmt = ps_pool.tile([128, DIM], f32, tag=f"dw{st % 2}", name="mtps")
            for ct in range(3):
                nc.tensor.matmul(psmt[:], A_cs[ct][:, 128 * st:128 * st + 128],
                                 wprojT[ct][:], start=(ct == 0), stop=(ct == 2))
            evac(mt8[:, st, :], psmt[:], S_MT)

        # ---- out0 = MT^T V (fp8 DR, lhsT constant across chunks) ----
        out0f8 = pp.tile([128, 3, NL], fp8, tag="u0", name=f"out0{b}")
        for mo in range(3):
            psos = [ps_pool.tile([128, CH], f32, tag=f"dw{ck}", name="avps")
                    for ck in range(NCHUNK)]
            for ck in range(NCHUNK):
                nc.tensor.matmul(psos[ck][:], mt8[:, 0:2, 128 * mo:128 * mo + 128],
                                 vcm8[b][:, 0:2, ck * CH:(ck + 1) * CH],
                                 perf_mode=DR, start=True, stop=False)
            for ck in range(NCHUNK):
                nc.tensor.matmul(psos[ck][:], mt8[:, 2, 128 * mo:128 * mo + 128],
                                 vcm8[b][:, 2, ck * CH:(ck + 1) * CH],
                                 start=False, stop=True)
            for ck in range(NCHUNK):
                evac(out0f8[:, mo, ck * CH:(ck + 1) * CH], psos[ck][:], EV_O)

        # ---- prompt branches; activations grouped by function ----
        g16a = pp.tile([128, 6, NL], bf16, tag="spm", name=f"g16{b}")
        for bi, br in enumerate(("chr", "detg")):   # all GELUs together
            for mo in range(3):
                psgs = [ps_pool.tile([128, CH], f32, tag=f"dw{ck}", name="gps")
                        for ck in range(NCHUNK)]
                for ck in range(NCHUNK):
                    nc.tensor.matmul(psgs[ck][:], w1T8[br][:, 0:2, 128 * mo:128 * mo + 128],
                                     out0f8[:, 0:2, ck * CH:(ck + 1) * CH],
                                     perf_mode=DR, start=True, stop=False)
                for ck in range(NCHUNK):
                    nc.tensor.matmul(psgs[ck][:], w1T8[br][:, 2, 128 * mo:128 * mo + 128],
                                     out0f8[:, 2, ck * CH:(ck + 1) * CH],
                                     start=False, stop=True)
                for ck in range(NCHUNK):
                    nc.scalar.activation(g16a[:, 3 * bi + mo, ck * CH:(ck + 1) * CH],
                                         psgs[ck][:], AF.Gelu,
                                         bias=b1[br][:, mo:mo + 1], scale=EV_G)
        gate16 = {}
        for bi, br in enumerate(("chr", "detg")):   # all sigmoids together
            gate16[br] = sp.tile([1, NL], bf16, tag=f"gate{bi}", name=f"gate{br}")
            for ck in range(NCHUNK):
                psgt = ps_pool.tile([16, CH], f32, tag=f"dw{ck % 4}", name="gateps")
                for kt in range(3):
                    nc.tensor.matmul(psgt[:], w2p[br][:, kt, :],
                                     g16a[:, 3 * bi + kt, ck * CH:(ck + 1) * CH],
                                     start=(kt == 0), stop=(kt == 2))
                nc.scalar.activation(gate16[br][:, ck * CH:(ck + 1) * CH], psgt[0:1, :],
                                     AF.Sigmoid, bias=b2sb[0:1, bi:bi + 1], scale=1.0)
        gated16 = {}
        for bi, br in enumerate(("chr", "detg")):
            gated16[br] = sp.tile([64, NL], bf16, tag=f"gtd{bi}", name=f"gated{br}")
            for ck in range(NCHUNK):
                psgb = ps_pool.tile([64, CH], f32, tag=f"dw{(ck + 2) % 4}", name="gbps")
                nc.tensor.matmul(psgb[:], ones1b[:], gate16[br][0:1, ck * CH:(ck + 1) * CH],
                                 start=True, stop=True)
                nc.vector.scalar_tensor_tensor(
                    out=gated16[br][:, ck * CH:(ck + 1) * CH],
                    in0=gk_sb[(b, br)][:, ck * CH:(ck + 1) * CH],
                    scalar=1.0, in1=psgb[:], op0=OP.mult, op1=OP.mult)
        # ---- alpha broadcast (Identity only -- no act-table load) ----
        al16 = pp.tile([128, NL], bf16, tag="u7", name="al16")
        oma16 = pp.tile([128, NL], bf16, tag="bigA", name="oma16")
        for ck in range(NCHUNK):
            psal = ps_pool.tile([128, CH], f32, tag=f"dw{ck % 4}", name="alps")
            nc.tensor.matmul(psal[:], zrep16[:], gk_sb[(b, "detg")][:, ck * CH:(ck + 1) * CH],
                             start=True, stop=True)
            nc.scalar.copy(al16[:, ck * CH:(ck + 1) * CH], psal[:])
            nc.scalar.activation(oma16[:, ck * CH:(ck + 1) * CH], psal[:],
                                 AF.Identity, bias=1.0, scale=-1.0)

        # ---- wt conv + silu, blended per mo so the tail overlaps ----
        for mo in range(3):
            pr = {}
            for bi, br in enumerate(("chr", "detg")):
                pr[br] = pp.tile([128, NL], bf16, tag=f"u{1 + 2 * bi + (mo % 2)}",
                                 name=f"pr{br}")
                for ck in range(NCHUNK):
                    pst2 = ps_pool.tile([128, CH], f32, tag=f"dw{ck % 4}", name="transps")
                    nc.tensor.matmul(pst2[:], wtT[br][:, 128 * mo:128 * mo + 128],
                                     gated16[br][:, ck * CH:(ck + 1) * CH],
                                     start=True, stop=True)
                    nc.scalar.activation(pr[br][:, ck * CH:(ck + 1) * CH], pst2[:],
                                         AF.Silu, bias=bt[br][:, mo:mo + 1], scale=1.0)
            pa = pp.tile([128, NL], bf16, tag="u5", name="pa")
            nc.vector.tensor_mul(pa[:], pr["chr"][:], oma16[:])
            pb = pp.tile([128, NL], bf16, tag="u6", name="pb")
            # keep batch 0 off the gpsimd queue -- its chain would otherwise
            # wait behind the batch-1 AllReduce queued on the same engine
            if b == 0:
                nc.vector.tensor_mul(pb[:], pr["detg"][:], al16[:])
            else:
                nc.gpsimd.tensor_mul(pb[:], pr["detg"][:], al16[:])
            p12 = sp.tile([128, NL], bf16, tag="p12", name="p12")
            nc.vector.tensor_add(p12[:], pa[:], pb[:])
            fin = pp.tile([128, NL], f32, tag="spm", name="fin")
            nc.vector.scalar_tensor_tensor(
                out=fin[:], in0=out0f8[:, mo, :], scalar=1.0 / S_O, in1=p12[:],
                op0=OP.mult, op1=OP.add)
            nc.sync.dma_start(t["OUT"][b, mo], fin[:])


_PROG = None


def _program():
    global _PROG
    if _PROG is None:
        _PROG = build_program()
    return _PROG


def kernel(**inputs):
    from concourse.bass_utils import run_bass_kernel_spmd
    nc = _program()
    consts = prep_constants(inputs)
    maps = shard_inputs(inputs, consts)
    res = run_bass_kernel_spmd(nc, maps, list(range(NCORE)))
    out = np.empty((B, DIM, Himg, Wimg), np.float32)
    for ci in range(NCORE):
        o = res.results[ci]["OUT"].reshape(B, DIM, ROWS, Wimg)
        out[:, :, ROWS * ci:ROWS * (ci + 1), :] = o
    return out
